# revision 15
# baseline (speedup 1.0000x reference)
"""Trainium2 Bass kernel for nn_DemoRecModel (2-layer dense transformer encoder).

Contract: kernel(**inputs) takes FULL unsharded numpy inputs (as produced by
setup_inputs()) and returns the FULL output [B, L, H] float32.

Strategy: data-parallel over batch across 8 NeuronCores (32 seqs/core).
On-chip layout is feature-major (hidden dim on partitions, tokens on the free
dim); dense projections run as float32r matmuls (full PE rate at N>=256),
attention internals run in bf16 with fp32 PSUM accumulation.
"""

import sys

sys.path.insert(0, "/opt/trn_rl_repo")

import numpy as np
import ml_dtypes

import concourse.bass as bass
import concourse.mybir as mybir
import concourse.tile as tile
from concourse import bacc, bass_utils
from concourse.masks import make_identity

# model dims (fixed by the problem)
B, L, H, NH, NL, V, FF = 256, 200, 256, 4, 2, 50000, 1024
DH = H // NH
EPS = 1e-12
NCORES = 8
BL = B // NCORES          # 32 seqs per core
TOK = BL * L              # 6400 tokens per core
CH = 2 * L                # 400-token chunks (2 seqs) for dense matmuls / LN
NCH = TOK // CH           # 16
P = 128
FRAGS = [(0, 128), (128, L - 128)]  # per-seq token fragments (128 + 72)

f32 = mybir.dt.float32
f32r = mybir.dt.float32r
bf16 = mybir.dt.bfloat16
i32 = mybir.dt.int32

AF = mybir.ActivationFunctionType
OP = mybir.AluOpType


def _build_program(flags):
    """Build the Bass program. flags: dict of runtime simplification flags."""
    nc = bacc.Bacc(
        "TRN2",
        target_bir_lowering=False,
        debug=False,
        enable_asserts=False,
        num_devices=NCORES,
    )

    # ---- DRAM tensors ----
    ids_d = nc.dram_tensor("ids", [TOK], i32, kind="ExternalInput").ap()
    item_d = nc.dram_tensor("item_emb", [V, H], f32, kind="ExternalInput").ap()
    pos_d = nc.dram_tensor("pos_emb", [L, H], f32, kind="ExternalInput").ap()
    elw_d = nc.dram_tensor("emb_ln_w", [H], f32, kind="ExternalInput").ap()
    elb_d = nc.dram_tensor("emb_ln_b", [H], f32, kind="ExternalInput").ap()
    Wd = {}
    for l in range(NL):
        Wd[f"wq{l}"] = nc.dram_tensor(f"wq{l}", [H, H], f32r, kind="ExternalInput").ap()
        Wd[f"wk{l}"] = nc.dram_tensor(f"wk{l}", [H, H], f32r, kind="ExternalInput").ap()
        Wd[f"wv{l}"] = nc.dram_tensor(f"wv{l}", [H, H], f32r, kind="ExternalInput").ap()
        Wd[f"wo{l}"] = nc.dram_tensor(f"wo{l}", [H, H], bf16, kind="ExternalInput").ap()
        Wd[f"w1{l}"] = nc.dram_tensor(f"w1{l}", [H, FF], f32r, kind="ExternalInput").ap()
        Wd[f"w2{l}"] = nc.dram_tensor(f"w2{l}", [FF, H], bf16, kind="ExternalInput").ap()
        for nm, dim in [("bq", H), ("bk", H), ("bv", H), ("bo", H), ("b1", FF), ("b2", H)]:
            if flags[f"use_{nm}"]:
                Wd[f"{nm}{l}"] = nc.dram_tensor(f"{nm}{l}", [dim], f32, kind="ExternalInput").ap()
        for nm in ["alw", "alb", "flw", "flb"]:
            Wd[f"{nm}{l}"] = nc.dram_tensor(f"{nm}{l}", [H], f32, kind="ExternalInput").ap()
    ones_d = nc.dram_tensor("ones_c", [P, P], f32r, kind="ExternalInput").ap()
    out_d = nc.dram_tensor("out", [H, TOK], f32, kind="ExternalOutput").ap()
    dbg = {}
    if flags.get("debug"):
        dbg["emb"] = nc.dram_tensor("dbg_emb", [H, TOK], f32, kind="ExternalOutput").ap()
        dbg["am"] = nc.dram_tensor("dbg_am", [P, 2 * BL], f32, kind="ExternalOutput").ap()
        dbg["q"] = nc.dram_tensor("dbg_q", [H, TOK], bf16, kind="ExternalOutput").ap()
        dbg["k"] = nc.dram_tensor("dbg_k", [H, TOK], bf16, kind="ExternalOutput").ap()
        dbg["ct"] = nc.dram_tensor("dbg_ct", [H, TOK], bf16, kind="ExternalOutput").ap()
        dbg["x1"] = nc.dram_tensor("dbg_x1", [H, TOK], f32, kind="ExternalOutput").ap()

    with tile.TileContext(nc) as tc:
        import contextlib

        with contextlib.ExitStack() as ctx:
            consts = ctx.enter_context(tc.tile_pool(name="consts", bufs=1))
            xpool = ctx.enter_context(tc.tile_pool(name="x", bufs=1))
            qkpool = ctx.enter_context(tc.tile_pool(name="qk", bufs=1))
            wpool = ctx.enter_context(tc.tile_pool(name="w", bufs=1))
            vpool = ctx.enter_context(tc.tile_pool(name="v", bufs=3))
            epool = ctx.enter_context(tc.tile_pool(name="e", bufs=4))
            spool = ctx.enter_context(tc.tile_pool(name="s", bufs=2))
            embp = ctx.enter_context(tc.tile_pool(name="emb", bufs=3))
            h1pool = ctx.enter_context(tc.tile_pool(name="h1", bufs=1))
            psum = ctx.enter_context(tc.tile_pool(name="ps", bufs=2, space="PSUM"))

            # ---- constants ----
            ident = consts.tile([P, P], f32)
            make_identity(nc, ident[:])
            ones128 = consts.tile([P, P], f32r)
            nc.sync.dma_start(out=ones128[:], in_=ones_d[:])
            ones_row = ones128
            pos_sb = [consts.tile([P, H], f32, tag=f"pos{f}", name=f"pos{f}") for f in range(2)]
            nc.sync.dma_start(out=pos_sb[0][:], in_=pos_d[0:128, :])
            nc.sync.dma_start(out=pos_sb[1][: L - 128], in_=pos_d[128:L, :])
            elw = consts.tile([P, 2], f32)
            elb = consts.tile([P, 2], f32)
            nc.sync.dma_start(out=elw[:], in_=elw_d.rearrange("(t p) -> p t", p=P))
            nc.sync.dma_start(out=elb[:], in_=elb_d.rearrange("(t p) -> p t", p=P))
            amadd = consts.tile([P, 2 * BL], f32)
            idt_all = consts.tile([P, 2 * BL], i32)
            for s in range(BL):
                for f, (off, cnt) in enumerate(FRAGS):
                    nc.sync.dma_start(
                        out=idt_all[:cnt, 2 * s + f: 2 * s + f + 1],
                        in_=ids_d[s * L + off: s * L + off + cnt, None])
            eps_t = consts.tile([P, 1], f32)
            nc.vector.memset(eps_t[:], EPS)

            # persistent activation buffers
            xt = [[xpool.tile([P, CH], f32r, tag=f"x_{ht}_{c}", name=f"x_{ht}_{c}") for c in range(NCH)]
                  for ht in range(2)]
            qT = [qkpool.tile([P, TOK], bf16, tag=f"q{ht}", name=f"qT{ht}") for ht in range(2)]
            kT = [qkpool.tile([P, TOK], bf16, tag=f"k{ht}", name=f"kT{ht}") for ht in range(2)]
            cT = [qkpool.tile([P, TOK], bf16, tag=f"c{ht}", name=f"cT{ht}") for ht in range(2)]

            # ================= embedding =================
            for s in range(BL):
                for f, (off, cnt) in enumerate(FRAGS):
                    c = s // 2
                    ccol = (s % 2) * L + off  # column offset inside chunk c
                    idt = idt_all[:, 2 * s + f: 2 * s + f + 1]
                    emb = embp.tile([P, H], f32, tag="emb")
                    nc.gpsimd.indirect_dma_start(
                        out=emb[:cnt, :],
                        out_offset=None,
                        in_=item_d[:],
                        in_offset=bass.IndirectOffsetOnAxis(ap=idt[:cnt, :1], axis=0),
                    )
                    # key-pad additive mask column: (id<=0) * -1e4
                    nc.vector.tensor_scalar(
                        amadd[:cnt, 2 * s + f: 2 * s + f + 1], idt[:cnt, :],
                        0, -10000.0, op0=OP.is_le, op1=OP.mult)
                    # add positional embedding
                    nc.vector.tensor_tensor(
                        out=emb[:cnt, :], in0=emb[:cnt, :], in1=pos_sb[f][:cnt, :], op=OP.add)
                    # token-major LN stats
                    st6 = embp.tile([P, 6], f32, tag="st6")
                    mv = embp.tile([P, 2], f32, tag="mv")
                    nc.vector.bn_stats(out=st6[:cnt, :], in_=emb[:cnt, :])
                    nc.vector.bn_aggr(out=mv[:cnt, :], in_=st6[:cnt, :])
                    rsig = embp.tile([P, 1], f32, tag="rsig")
                    nbias = embp.tile([P, 1], f32, tag="nbias")
                    nc.scalar.activation(rsig[:cnt], mv[:cnt, 1:2], AF.Sqrt, bias=eps_t[:cnt])
                    nc.vector.reciprocal(rsig[:cnt], rsig[:cnt])
                    nc.vector.tensor_scalar(nbias[:cnt], mv[:cnt, 0:1], -1.0, None, op0=OP.mult)
                    nc.vector.tensor_tensor(out=nbias[:cnt], in0=nbias[:cnt], in1=rsig[:cnt], op=OP.mult)
                    z = embp.tile([P, H], f32, tag="z")
                    nc.scalar.activation(z[:cnt], emb[:cnt], AF.Identity,
                                         bias=nbias[:cnt], scale=rsig[:cnt])
                    for ht in range(2):
                        ps = psum.tile([P, P], f32, tag="att")
                        nc.tensor.transpose(ps[:, :cnt], z[:cnt, ht * P:(ht + 1) * P], ident[:cnt, :cnt])
                        dst = xt[ht][c][:, ccol:ccol + cnt]
                        if flags["emb_ln_trivial"]:
                            nc.scalar.copy(dst, ps[:, :cnt])
                        else:
                            nc.vector.tensor_scalar(
                                dst, ps[:, :cnt], elw[:, ht:ht + 1], elb[:, ht:ht + 1],
                                op0=OP.mult, op1=OP.add)

            if flags.get("debug"):
                for ht in range(2):
                    for c in range(NCH):
                        nc.sync.dma_start(out=dbg["emb"][ht * P:(ht + 1) * P, c * CH:(c + 1) * CH],
                                          in_=xt[ht][c][:].bitcast(f32))
                nc.sync.dma_start(out=dbg["am"][:], in_=amadd[:])

            # ================= encoder layers =================
            for l in range(NL):
                wq = [wpool.tile([P, H], f32r, tag=f"wq{k}", name=f"wq_{l}_{k}") for k in range(2)]
                wk = [wpool.tile([P, H], f32r, tag=f"wk{k}", name=f"wk_{l}_{k}") for k in range(2)]
                wv = [wpool.tile([P, H], f32r, tag=f"wv{k}", name=f"wv_{l}_{k}") for k in range(2)]
                wo = [wpool.tile([P, H], bf16, tag=f"wo{k}", name=f"wo_{l}_{k}") for k in range(2)]
                w1 = [wpool.tile([P, FF], f32r, tag=f"w1{k}", name=f"w1_{l}_{k}") for k in range(2)]
                w2 = [wpool.tile([P, H], bf16, tag=f"w2{k}", name=f"w2_{l}_{k}") for k in range(8)]
                for k in range(2):
                    nc.sync.dma_start(out=wq[k][:], in_=Wd[f"wq{l}"][k * P:(k + 1) * P, :])
                    nc.sync.dma_start(out=wk[k][:], in_=Wd[f"wk{l}"][k * P:(k + 1) * P, :])
                    nc.sync.dma_start(out=wv[k][:], in_=Wd[f"wv{l}"][k * P:(k + 1) * P, :])
                    nc.sync.dma_start(out=wo[k][:], in_=Wd[f"wo{l}"][k * P:(k + 1) * P, :])
                    nc.sync.dma_start(out=w1[k][:], in_=Wd[f"w1{l}"][k * P:(k + 1) * P, :])
                for k in range(8):
                    nc.sync.dma_start(out=w2[k][:], in_=Wd[f"w2{l}"][k * P:(k + 1) * P, :])
                bias_t = {}
                for nm, nt in [("bq", 2), ("bk", 2), ("bv", 2), ("bo", 2), ("b1", 8), ("b2", 2)]:
                    if flags[f"use_{nm}"]:
                        t = wpool.tile([P, nt], f32, tag=f"{nm}t")
                        nc.sync.dma_start(out=t[:], in_=Wd[f"{nm}{l}"].rearrange("(t p) -> p t", p=P))
                        bias_t[nm] = t
                ln_t = {}
                for nm in ["alw", "alb", "flw", "flb"]:
                    t = wpool.tile([P, 2], f32, tag=f"{nm}t")
                    nc.sync.dma_start(out=t[:], in_=Wd[f"{nm}{l}"].rearrange("(t p) -> p t", p=P))
                    ln_t[nm] = t
                bv_row = None
                if flags["use_bv"]:
                    bv_row = wpool.tile([1, H], f32r, tag="bvrow")
                    nc.sync.dma_start(out=bv_row[:], in_=Wd[f"bv{l}"][None, :])

                # ---- Q / K projections (feature-major, bf16 outputs) ----
                for wsb, dstT, bnm in [(wq, qT, "bq"), (wk, kT, "bk")]:
                    for m in range(2):
                        for c in range(NCH):
                            ps = psum.tile([P, CH], f32, tag="mm")
                            for kk in range(2):
                                nc.tensor.matmul(
                                    ps[:], (wsb[kk][:, m * P:(m + 1) * P]), (xt[kk][c][:]),
                                    start=(kk == 0), stop=(kk == 1))
                            dst = dstT[m][:, c * CH:(c + 1) * CH]
                            if flags[f"use_{bnm}"]:
                                nc.scalar.activation(dst, ps[:], AF.Identity,
                                                     bias=bias_t[bnm][:, m:m + 1])
                            else:
                                nc.scalar.copy(dst, ps[:])

                # ---- per-seq: V projection + attention ----
                for s in range(BL):
                    c2 = s // 2
                    scol = (s % 2) * L  # seq column offset inside chunk c2
                    vt = vpool.tile([P, 2, 4, 65], bf16, tag="vt")
                    for f, (off, cnt) in enumerate(FRAGS):
                        ps = psum.tile([P, H], f32, tag="mm")
                        for kk in range(2):
                            nc.tensor.matmul(
                                ps[:cnt, :], (xt[kk][c2][:, scol + off:scol + off + cnt]),
                                (wv[kk][:]),
                                start=(kk == 0), stop=(kk == 1 and not flags["use_bv"]))
                        if flags["use_bv"]:
                            nc.tensor.matmul(ps[:cnt, :], (ones_row[:1, :cnt]), (bv_row[:]),
                                             start=False, stop=True)
                        nc.scalar.copy(
                            vt[:cnt, f, :, 0:64],
                            ps[:cnt, :].rearrange("p (h d) -> p h d", h=NH))
                        nc.vector.memset(vt[:cnt, f, :, 64:65], 1.0)
                    for h in range(NH):
                        kti, krow = h // 2, (h % 2) * 64
                        cps = psum.tile([P, L], f32, tag="ctx")
                        for f, (off, cnt) in enumerate(FRAGS):
                            sps = psum.tile([P, L], f32, tag="att")
                            nc.tensor.matmul(
                                sps[:cnt, :],
                                kT[kti][krow:krow + 64, s * L + off: s * L + off + cnt],
                                qT[kti][krow:krow + 64, s * L: s * L + L],
                                start=True, stop=True)
                            et = epool.tile([P, L], bf16, tag="et")
                            nc.scalar.activation(
                                et[:cnt, :], sps[:cnt, :], AF.Exp,
                                bias=amadd[:cnt, 2 * s + f: 2 * s + f + 1], scale=0.125)
                            nc.gpsimd.affine_select(
                                et[:cnt, :], et[:cnt, :], pattern=[[1, L]],
                                compare_op=OP.is_ge, fill=0.0,
                                base=-off, channel_multiplier=-1)
                            nc.tensor.matmul(
                                cps[:65, :], vt[:cnt, f, h, :], et[:cnt, :],
                                start=(f == 0), stop=(f == 1))
                        rs = spool.tile([1, L], f32, tag="rs")
                        nc.vector.reciprocal(rs[:], cps[64:65, :])
                        rb = spool.tile([64, L], f32, tag="rb")
                        nc.gpsimd.partition_broadcast(rb[:], rs[:])
                        nc.vector.tensor_tensor(
                            out=cT[kti][krow:krow + 64, s * L:(s + 1) * L],
                            in0=cps[0:64, :], in1=rb[:], op=OP.mult)

                if flags.get("debug") and l == 0:
                    for ht in range(2):
                        nc.sync.dma_start(out=dbg["q"][ht * P:(ht + 1) * P, :], in_=qT[ht][:])
                        nc.sync.dma_start(out=dbg["k"][ht * P:(ht + 1) * P, :], in_=kT[ht][:])
                        nc.sync.dma_start(out=dbg["ct"][ht * P:(ht + 1) * P, :], in_=cT[ht][:])

                # ---- output projection + residual + LN ; then FFN + residual + LN ----
                for c in range(NCH):
                    for m in range(2):
                        ps = psum.tile([P, CH], f32, tag="mm")
                        for kk in range(2):
                            nc.tensor.matmul(ps[:], wo[kk][:, m * P:(m + 1) * P],
                                             cT[kk][:, c * CH:(c + 1) * CH],
                                             start=(kk == 0), stop=(kk == 1))
                        if flags["use_bo"]:
                            nc.vector.tensor_scalar(ps[:], ps[:], bias_t["bo"][:, m:m + 1],
                                                    None, op0=OP.add)
                        nc.vector.tensor_tensor(out=xt[m][c][:], in0=ps[:], in1=xt[m][c][:],
                                                op=OP.add)
                    self_ln(nc, psum, spool, xt, c, ones128, eps_t, ln_t["alw"], ln_t["alb"],
                            flags["aln_trivial"])
                    if flags.get("debug") and l == 0:
                        for ht in range(2):
                            nc.sync.dma_start(out=dbg["x1"][ht * P:(ht + 1) * P, c * CH:(c + 1) * CH],
                                              in_=xt[ht][c][:].bitcast(f32))

                    # FFN on this chunk
                    h1 = []
                    for fm in range(8):
                        ps = psum.tile([P, CH], f32, tag="mm")
                        for kk in range(2):
                            nc.tensor.matmul(ps[:], (w1[kk][:, fm * P:(fm + 1) * P]),
                                             (xt[kk][c][:]),
                                             start=(kk == 0), stop=(kk == 1))
                        ht_ = h1pool.tile([P, CH], bf16, tag=f"h1_{fm}")
                        if flags["use_b1"]:
                            nc.scalar.activation(ht_[:], ps[:], AF.Gelu,
                                                 bias=bias_t["b1"][:, fm:fm + 1])
                        else:
                            nc.scalar.activation(ht_[:], ps[:], AF.Gelu)
                        h1.append(ht_)
                    for m in range(2):
                        ps = psum.tile([P, CH], f32, tag="mm")
                        for kf in range(8):
                            nc.tensor.matmul(ps[:], w2[kf][:, m * P:(m + 1) * P], h1[kf][:],
                                             start=(kf == 0), stop=(kf == 7))
                        if flags["use_b2"]:
                            nc.vector.tensor_scalar(ps[:], ps[:], bias_t["b2"][:, m:m + 1],
                                                    None, op0=OP.add)
                        nc.vector.tensor_tensor(out=xt[m][c][:], in0=ps[:], in1=xt[m][c][:],
                                                op=OP.add)
                    self_ln(nc, psum, spool, xt, c, ones128, eps_t, ln_t["flw"], ln_t["flb"],
                            flags["fln_trivial"])

                    if l == NL - 1:
                        for ht in range(2):
                            nc.sync.dma_start(
                                out=out_d[ht * P:(ht + 1) * P, c * CH:(c + 1) * CH],
                                in_=xt[ht][c][:].bitcast(f32))

    nc.compile()
    return nc


def self_ln(nc, psum, spool, xt, c, ones128, eps_t, w_t, b_t, trivial):
    """Feature-major LayerNorm (in-place) on chunk c of the residual stream."""
    sq = [spool.tile([P, CH], f32r, tag=f"sq{ht}", name=f"sq{ht}") for ht in range(2)]
    for ht in range(2):
        nc.scalar.square(sq[ht][:], xt[ht][c][:])
    sumps = psum.tile([P, CH], f32, tag="st")
    sqps = psum.tile([P, CH], f32, tag="st")
    for ht in range(2):
        nc.tensor.matmul(sumps[:], (ones128[:]), (xt[ht][c][:]),
                         start=(ht == 0), stop=(ht == 1))
    for ht in range(2):
        nc.tensor.matmul(sqps[:], (ones128[:]), (sq[ht][:]),
                         start=(ht == 0), stop=(ht == 1))
    muB = spool.tile([P, CH], f32, tag="muB")
    nc.vector.tensor_scalar(muB[:], sumps[:], 1.0 / H, None, op0=OP.mult)
    m2 = spool.tile([P, CH], f32, tag="m2")
    nc.scalar.square(m2[:], muB[:])
    varB = spool.tile([P, CH], f32, tag="varB")
    nc.vector.tensor_scalar(varB[:], sqps[:], 1.0 / H, None, op0=OP.mult)
    nc.vector.tensor_tensor(out=varB[:], in0=varB[:], in1=m2[:], op=OP.subtract)
    nc.scalar.activation(varB[:], varB[:], AF.Sqrt, bias=eps_t[:])
    nc.vector.reciprocal(varB[:], varB[:])
    for ht in range(2):
        nc.vector.tensor_tensor(out=xt[ht][c][:], in0=xt[ht][c][:], in1=muB[:],
                                op=OP.subtract)
        nc.vector.tensor_tensor(out=xt[ht][c][:], in0=xt[ht][c][:], in1=varB[:],
                                op=OP.mult)
        if not trivial:
            nc.vector.tensor_scalar(xt[ht][c][:], xt[ht][c][:], w_t[:, ht:ht + 1],
                                    b_t[:, ht:ht + 1], op0=OP.mult, op1=OP.add)


_PROGRAM_CACHE = {}
LAST_RESULT = None
DEBUG = False


def kernel(**inputs):
    item_emb = np.asarray(inputs["item_emb"], dtype=np.float32)
    pos_emb = np.asarray(inputs["pos_emb"], dtype=np.float32)
    emb_ln_w = np.asarray(inputs["emb_ln_w"], dtype=np.float32)
    emb_ln_b = np.asarray(inputs["emb_ln_b"], dtype=np.float32)
    ids = np.asarray(inputs["input_ids"]).astype(np.int32)  # [B, L]
    Wq = np.asarray(inputs["Wq"], dtype=np.float32)
    Wk = np.asarray(inputs["Wk"], dtype=np.float32)
    Wv = np.asarray(inputs["Wv"], dtype=np.float32)
    Wo = np.asarray(inputs["Wo"], dtype=np.float32)
    W1 = np.asarray(inputs["W1"], dtype=np.float32)
    W2 = np.asarray(inputs["W2"], dtype=np.float32)
    bq = np.asarray(inputs["bq"], dtype=np.float32)
    bk = np.asarray(inputs["bk"], dtype=np.float32)
    bv = np.asarray(inputs["bv"], dtype=np.float32)
    bo = np.asarray(inputs["bo"], dtype=np.float32)
    b1 = np.asarray(inputs["b1"], dtype=np.float32)
    b2 = np.asarray(inputs["b2"], dtype=np.float32)
    alw = np.asarray(inputs["attn_ln_w"], dtype=np.float32)
    alb = np.asarray(inputs["attn_ln_b"], dtype=np.float32)
    flw = np.asarray(inputs["ffn_ln_w"], dtype=np.float32)
    flb = np.asarray(inputs["ffn_ln_b"], dtype=np.float32)

    flags = {
        "debug": DEBUG,
        "use_bq": bool(np.any(bq)), "use_bk": bool(np.any(bk)),
        "use_bv": bool(np.any(bv)), "use_bo": bool(np.any(bo)),
        "use_b1": bool(np.any(b1)), "use_b2": bool(np.any(b2)),
        "emb_ln_trivial": bool(np.all(emb_ln_w == 1) and not np.any(emb_ln_b)),
        "aln_trivial": bool(np.all(alw == 1) and not np.any(alb)),
        "fln_trivial": bool(np.all(flw == 1) and not np.any(flb)),
    }
    key = tuple(sorted(flags.items()))
    if key not in _PROGRAM_CACHE:
        _PROGRAM_CACHE[key] = _build_program(flags)
    nc = _PROGRAM_CACHE[key]

    shared = {
        "ones_c": np.ones((128, 128), dtype=np.float32),
        "item_emb": item_emb, "pos_emb": pos_emb,
        "emb_ln_w": emb_ln_w, "emb_ln_b": emb_ln_b,
    }
    for l in range(NL):
        shared[f"wq{l}"] = np.ascontiguousarray(Wq[l])
        shared[f"wk{l}"] = np.ascontiguousarray(Wk[l])
        shared[f"wv{l}"] = np.ascontiguousarray(Wv[l])
        shared[f"wo{l}"] = np.ascontiguousarray(Wo[l]).astype(ml_dtypes.bfloat16)
        shared[f"w1{l}"] = np.ascontiguousarray(W1[l])
        shared[f"w2{l}"] = np.ascontiguousarray(W2[l]).astype(ml_dtypes.bfloat16)
        for nm, arr in [("bq", bq), ("bk", bk), ("bv", bv), ("bo", bo),
                        ("b1", b1), ("b2", b2)]:
            shared[f"{nm}{l}"] = np.ascontiguousarray(arr[l])
        shared[f"alw{l}"] = np.ascontiguousarray(alw[l])
        shared[f"alb{l}"] = np.ascontiguousarray(alb[l])
        shared[f"flw{l}"] = np.ascontiguousarray(flw[l])
        shared[f"flb{l}"] = np.ascontiguousarray(flb[l])

    in_maps = []
    for core in range(NCORES):
        m = dict(shared)
        m["ids"] = np.ascontiguousarray(ids[core * BL:(core + 1) * BL].reshape(-1))
        in_maps.append(m)

    res = bass_utils.run_bass_kernel_spmd(nc, in_maps, core_ids=list(range(NCORES)))
    global LAST_RESULT
    LAST_RESULT = res
    outs = []
    for core in range(NCORES):
        o = res.results[core]["out"]  # [H, TOK] feature-major
        outs.append(np.ascontiguousarray(o.T).reshape(BL, L, H))
    return np.concatenate(outs, axis=0)
